# revision 28
# baseline (speedup 1.0000x reference)
"""GAT layer on 8 Trainium2 NeuronCores (Bass/Tile) — v2.

Strategy (target-per-partition, edge gathers via ANT dma_gather):
  - Nodes split into 2 balanced halves (greedy: each target's edges split
    ~deg/2 per half) laid out as table rows [half0 | sent0 | half1 | sent1].
    Each gather call uses a signed-int16 index window (base row per call,
    idx = row - base in [-32768, 32767]) so one call covers a whole half.
  - Targets sorted globally by max-half count, dealt round-robin to the 8
    cores, so all cores share one tight width schedule W[block, half].
  - Blocks grouped into batches (<=4 blocks, bounded G tile width) with
    batch-uniform widths; one gather call per (batch, half).
  - Table rows (512B): [p bf16(128) | alpha_src f32(8) | pad]. Built by PE
    matmuls (stationary xtt tile f32, moving [W_proj | W_proj@Ablk]).
    Sentinel rows come from a host-solved x column with Wa^T x = -300, so
    exp(s) underflows to 0 for padding slots.
  - Per batch: beta/skip via PE from xTperm; s = alpha + beta; factored
    exp(lrelu) on ACT; E-weighted sum U and denominator D via DVE strided
    reduces; out = U/D + skip, ELU.
"""

import os
import sys

sys.path.insert(0, "/opt/trn_rl_repo")

import numpy as np
from contextlib import ExitStack

import concourse.bass as bass
import concourse.bacc as bacc
import concourse.tile as tile
from concourse import mybir
from concourse._compat import cdiv
from concourse.bass_utils import run_bass_kernel_spmd
from concourse.library_config import mlp

N_NODES = 100000
N_EDGES = 1600000
IN_F = 128
H = 8
F = 16
HF = H * F  # 128
NEG_SLOPE = 0.2
EPS = 1e-16
N_CORES = 8
TGT_PER_CORE = N_NODES // N_CORES  # 12500
N_BLOCKS = cdiv(TGT_PER_CORE, 128)  # 98
TGT_PAD = N_BLOCKS * 128  # 12544
NH = 2
HALF_CAP = 65000
ROW_ELEMS = 256  # bf16 elems per table row (512B)
TABLE_ROWS = N_NODES + 2  # 100002 (two sentinels)
TABLE_ROWS_PAD = cdiv(TABLE_ROWS, 1024) * 1024  # staging writes full groups
ALPHA_SENT = -300.0
GCOL_LIMIT = 64  # max gather-tile columns per batch (SBUF budget)

_COMPILED = {}


def _host_prep(x, edge_index, W_proj, W_skip, a_src, a_tgt):
    x = np.asarray(x, np.float32)
    ei = np.asarray(edge_index)
    src = ei[0].astype(np.int64)
    tgt = ei[1].astype(np.int64)

    Wp = np.asarray(W_proj, np.float32)
    Ws = np.asarray(W_skip, np.float32)
    asr = np.asarray(a_src, np.float32).reshape(H, F)
    atg = np.asarray(a_tgt, np.float32).reshape(H, F)
    # block-diagonal score layouts: Ablk[hf, h] = a_src[h, f'] at hf=h*16+f'
    Ablk = np.zeros((HF, H), np.float32)
    Bblk = np.zeros((HF, H), np.float32)
    for h in range(H):
        Ablk[h * F:(h + 1) * F, h] = asr[h]
        Bblk[h * F:(h + 1) * F, h] = atg[h]
    Wa = Wp @ Ablk  # [128, 8]
    Wb = Wp @ Bblk
    pack0 = np.concatenate([Wp, Wa], axis=1)  # [128, 136]
    pack2 = np.concatenate([Ws, Wb], axis=1)

    # sentinel x column: Wa^T xs = ALPHA_SENT * 1  (min-norm solution)
    xs = np.linalg.lstsq(Wa.T, np.full(H, ALPHA_SENT, np.float64),
                         rcond=None)[0].astype(np.float32)

    # ---- balanced half assignment (per-target even split) ----
    eorder = np.argsort(src, kind="stable")
    t_by_src = tgt[eorder]
    starts = np.searchsorted(src[eorder], np.arange(N_NODES + 1))
    deg_out = np.diff(starts)
    cnt = np.zeros((N_NODES, NH), np.int32)
    hof = np.zeros(N_NODES, np.int8)
    hsize = np.zeros(NH, np.int64)
    for n in np.argsort(-deg_out, kind="stable"):
        ts = t_by_src[starts[n]:starts[n + 1]]
        if len(ts):
            sc = cnt[ts].sum(axis=0).astype(np.float64)
        else:
            sc = hsize.astype(np.float64) * 1e-6
        sc[hsize >= HALF_CAP] = np.inf
        q = int(np.argmin(sc))
        hof[n] = q
        hsize[q] += 1
        if len(ts):
            np.add.at(cnt, (ts, q), 1)
    # refinement sweeps: move nodes between halves when it lowers
    # sum_t max(cnt[t,0], cnt[t,1])
    for _sweep in range(2):
        moved = 0
        for n in np.argsort(-deg_out, kind="stable"):
            ts = t_by_src[starts[n]:starts[n + 1]]
            if not len(ts):
                continue
            q = int(hof[n])
            qn = 1 - q
            if hsize[qn] >= HALF_CAP:
                continue
            c = cnt[ts]  # [k, 2]
            cur = np.maximum(c[:, 0], c[:, 1])
            cq = c[:, q].copy()
            cqn = c[:, qn].copy()
            new = np.maximum(cq - 1, cqn + 1)
            delta = int((new - cur).sum())
            if delta < 0:
                np.add.at(cnt, (ts, q), -1)
                np.add.at(cnt, (ts, qn), 1)
                hof[n] = qn
                hsize[q] -= 1
                hsize[qn] += 1
                moved += 1
        if moved == 0:
            break
    n0 = int(hsize[0])
    n1 = int(hsize[1])
    assert n0 + n1 == N_NODES and max(n0, n1) <= HALF_CAP
    assert n0 + 1 >= (N_NODES + 1 - 32767) - 32768, \
        "half1 window out of int16 range"

    # table rows: [half0 nodes | sent0 | half1 nodes | sent1]
    tabrow = np.empty(N_NODES, np.int64)
    h0_nodes = np.flatnonzero(hof == 0)
    h1_nodes = np.flatnonzero(hof == 1)
    tabrow[h0_nodes] = np.arange(n0)
    tabrow[h1_nodes] = n0 + 1 + np.arange(n1)
    sent_row = [n0, N_NODES + 1]
    base = [max(0, n0 - 32767), N_NODES + 1 - 32767]
    assert -32768 <= 0 - base[0] and sent_row[0] - base[0] <= 32767
    assert -32768 <= (n0 + 1) - base[1] and sent_row[1] - base[1] <= 32767

    # xTtab: x columns in table-row order (bf16), sentinel cols = xs
    import ml_dtypes
    bf = ml_dtypes.bfloat16
    xTtab = np.empty((IN_F, TABLE_ROWS), bf)
    xTtab[:, :n0] = x[h0_nodes].T
    xTtab[:, n0] = xs
    xTtab[:, n0 + 1:n0 + 1 + n1] = x[h1_nodes].T
    xTtab[:, N_NODES + 1] = xs

    # ---- global target ranking & shared width schedule ----
    mx = cnt.max(axis=1)
    rank = np.argsort(-mx, kind="stable")  # rank r -> target node
    rank_of = np.empty(N_NODES, np.int64)
    rank_of[rank] = np.arange(N_NODES)
    cnt_pad = np.vstack([cnt[rank],
                         np.zeros((N_BLOCKS * 1024 - N_NODES, NH), np.int32)])
    Wblk = cnt_pad.reshape(N_BLOCKS, 1024, NH).max(axis=1)  # [98, 2]

    # batches: <=4 blocks, nblk*(W0+W1) <= GCOL_LIMIT
    batches = []  # (b0, nblk, W0, W1, x0, x1)
    b = 0
    while b < N_BLOCKS:
        nblk = 1
        W0, W1 = int(Wblk[b, 0]), int(Wblk[b, 1])
        while (b + nblk < N_BLOCKS and nblk < 4):
            nW0 = max(W0, int(Wblk[b + nblk, 0]))
            nW1 = max(W1, int(Wblk[b + nblk, 1]))
            if (nblk + 1) * (nW0 + nW1) > GCOL_LIMIT:
                break
            W0, W1 = nW0, nW1
            nblk += 1
        # extra all-sentinel column needed only if the stream's last slot
        # (last block, partition 127, last depth) can be a real edge on
        # some core (firmware trims trailing negative idxs)
        j_last = (b + nblk - 1) * 128 + 127
        xs_ = []
        for hh, Wh in ((0, W0), (1, W1)):
            cmax = int(cnt_pad[j_last * 8:j_last * 8 + 8, hh].max())
            xs_.append(1 if (Wh > 0 and cmax >= Wh) else 0)
        batches.append((b, nblk, W0, W1, xs_[0], xs_[1]))
        b += nblk

    # ---- per-core edge slots ----
    core_of = rank_of[tgt] % N_CORES
    j_of = rank_of[tgt] // N_CORES  # 0..12499 within core
    h_of_e = hof[src].astype(np.int64)
    idxval = (tabrow[src] - np.array(base)[h_of_e]).astype(np.int64)

    per_core = []
    for c in range(N_CORES):
        m = core_of == c
        j = j_of[m]
        he = h_of_e[m]
        iv = idxval[m]
        # occurrence index within each (j, h) group
        eo = np.lexsort((np.zeros_like(j), he, j))
        j_s, h_s, iv_s = j[eo], he[eo], iv[eo]
        key = j_s * NH + h_s
        uk, firsts = np.unique(key, return_index=True)
        d_s = np.arange(len(key)) - firsts[np.searchsorted(uk, key)]
        blk_s = j_s // 128
        p_s = j_s % 128

        idx_cols = []
        for bi, (b0, nblk, W0, W1, x0, x1) in enumerate(batches):
            mm_b = (blk_s >= b0) & (blk_s < b0 + nblk)
            for hh, Wh, xh in ((0, W0, x0), (1, W1, x1)):
                if Wh == 0:
                    continue
                arr = np.full((nblk * Wh + xh, 128),
                              sent_row[hh] - base[hh], np.int64)
                mm = mm_b & (h_s == hh)
                if mm.any():
                    col = (blk_s[mm] - b0) * Wh + d_s[mm]
                    arr[col, p_s[mm]] = iv_s[mm]
                flat = arr.reshape(-1)  # j = col*128 + p
                assert flat[-1] >= 0
                wrap = flat.reshape(-1, 16).T  # [16, ni/16]
                idx_cols.append(np.tile(wrap, (8, 1)))
        idxs = np.concatenate(idx_cols, axis=1).astype(np.int16)

        # xTperm: x columns of this core's targets in rank order
        perm = rank[np.arange(TGT_PER_CORE) * N_CORES + c]  # j -> node
        xTp = np.zeros((IN_F, TGT_PAD), bf)
        xTp[:, :TGT_PER_CORE] = x[perm].T
        per_core.append(dict(idxs=np.ascontiguousarray(idxs),
                             xTperm=np.ascontiguousarray(xTp), perm=perm))

    pack0 = pack0.astype(bf)
    pack2 = pack2.astype(bf)
    common = dict(xTtab=xTtab, pack0=pack0, pack2=pack2,
                  batches=batches, base=base, n0=n0,
                  C_total=per_core[0]["idxs"].shape[1])
    for pc in per_core:
        assert pc["idxs"].shape[1] == common["C_total"]
    return common, per_core


def _build_program(batches, base, C_total, n0):
    nc = bacc.Bacc("TRN2", debug=False, num_devices=N_CORES,
                   num_swdge_queues=4)
    f32 = mybir.dt.float32
    bf16 = mybir.dt.bfloat16
    i16 = mybir.dt.int16

    xTtab_d = nc.dram_tensor("xTtab", [IN_F, TABLE_ROWS], bf16,
                             kind="ExternalInput").ap()
    xTperm_d = nc.dram_tensor("xTperm", [IN_F, TGT_PAD], bf16,
                              kind="ExternalInput").ap()
    pack0_d = nc.dram_tensor("pack0", [IN_F, HF + H], bf16,
                             kind="ExternalInput").ap()
    pack2_d = nc.dram_tensor("pack2", [IN_F, HF + H], bf16,
                             kind="ExternalInput").ap()
    idxs_d = nc.dram_tensor("idxs", [128, C_total], i16,
                            kind="ExternalInput").ap()
    out_d = nc.dram_tensor("out", [TGT_PAD, HF], f32,
                           kind="ExternalOutput").ap()
    table = nc.dram_tensor("table", [TABLE_ROWS_PAD, ROW_ELEMS], bf16).ap()

    with tile.TileContext(nc) as tc, ExitStack() as ctx:
        consts = ctx.enter_context(tc.tile_pool(name="consts", bufs=1))
        stg = ctx.enter_context(tc.tile_pool(name="stg", bufs=3))
        gpool = ctx.enter_context(tc.tile_pool(name="gpool", bufs=3))
        work = ctx.enter_context(tc.tile_pool(name="work", bufs=2))
        epi = ctx.enter_context(tc.tile_pool(name="epi", bufs=2))
        psum = ctx.enter_context(tc.tile_pool(name="psum", bufs=3,
                                              space="PSUM"))
        psum2 = ctx.enter_context(tc.tile_pool(name="psum2", bufs=2,
                                               space="PSUM"))
        idxp = ctx.enter_context(tc.tile_pool(name="idxp", bufs=2))

        nc.gpsimd.load_library(mlp)

        pack0 = consts.tile([IN_F, HF + H], bf16)
        nc.sync.dma_start(out=pack0[:], in_=pack0_d[:])
        pack2 = consts.tile([IN_F, HF + H], bf16)
        nc.sync.dma_start(out=pack2[:], in_=pack2_d[:])

        # --- Phase B: build table (groups of 16 row-tiles = 2048 rows) ---
        from concourse.tile_rust import add_dep_helper
        tab_stores = [[], []]  # store insts overlapping each half's rows
        GR = 2048
        NGRP = TABLE_ROWS_PAD // GR
        for g in range(NGRP):
            r0 = g * GR
            ncols = min(GR, TABLE_ROWS - r0)  # real columns to load
            nt = cdiv(ncols, 128)
            xtt = stg.tile([IN_F, GR], bf16, tag="xtt")
            nc.sync.dma_start(out=xtt[:, :ncols],
                              in_=xTtab_d[:, r0:r0 + ncols])
            rows_s = stg.tile([128, GR // 128, ROW_ELEMS], bf16, tag="rows")
            for t in range(nt):
                nr = min(128, ncols - t * 128)
                pa = psum.tile([128, HF + H], f32, space="PSUM", tag="pa")
                nc.tensor.matmul(out=pa[:nr],
                                 lhsT=xtt[:, t * 128:t * 128 + nr],
                                 rhs=pack0[:], start=True, stop=True)
                if t % 2 == 0:
                    nc.scalar.activation(
                        out=rows_s[:nr, t, 0:HF], in_=pa[:nr, 0:HF],
                        func=mybir.ActivationFunctionType.Copy)
                else:
                    nc.vector.tensor_copy(out=rows_s[:nr, t, 0:HF],
                                          in_=pa[:nr, 0:HF])
                nc.vector.tensor_copy(
                    out=rows_s[:nr, t, HF:HF + 2 * H].bitcast(f32),
                    in_=pa[:nr, HF:HF + H])
            # store: table row r0 + t*128 + p <- rows_s[p, t, :]
            tab_view = table[r0:r0 + GR, :].rearrange(
                "(t p) e -> p t e", t=GR // 128)
            st = nc.sync.dma_start(out=tab_view, in_=rows_s[:])
            if r0 <= n0:  # overlaps window A rows [0, n0]
                tab_stores[0].append(st)
            if r0 + GR > n0 + 1:  # overlaps window B rows [n0+1, end]
                tab_stores[1].append(st)

        # table-half-complete anchors: the gather in_ap only declares one
        # row, so the dep tracker can't see the real read range — add
        # explicit edges table-stores -> anchor -> gather.
        anchors = []
        for hh in range(NH):
            a = nc.sync.nop()
            for st in tab_stores[hh]:
                add_dep_helper(a.ins, st.ins, sync=True,
                               reason="table half complete")
            anchors.append(a)

        # --- Phase C: batched gathers + compute ---
        col_off = 0
        call_i = 0
        for bi, (b0, nblk, W0, W1, x0, x1) in enumerate(batches):
            sw = nblk * (W0 + W1)
            # beta / skip
            xp = stg.tile([IN_F, nblk * 128], bf16, tag="xp")
            nc.sync.dma_start(
                out=xp[:], in_=xTperm_d[:, b0 * 128:(b0 + nblk) * 128])
            sk = epi.tile([128, nblk, HF + H], f32, tag="sk")
            for k in range(nblk):
                sk_ps = psum2.tile([128, HF + H], f32, space="PSUM",
                                   tag="skps")
                nc.tensor.matmul(out=sk_ps[:],
                                 lhsT=xp[:, k * 128:(k + 1) * 128],
                                 rhs=pack2[:], start=True, stop=True)
                nc.scalar.activation(out=sk[:, k, :], in_=sk_ps[:],
                                     func=mybir.ActivationFunctionType.Copy)

            # idx + gathers (optional extra all-sentinel column per call)
            nex = x0 + x1
            Cb = (nblk * (W0 + W1) + nex) * 8
            idx_t = idxp.tile([128, Cb], i16, tag="idxg")
            nc.sync.dma_start(out=idx_t[:],
                              in_=idxs_d[:, col_off:col_off + Cb])
            col_off += Cb

            G = gpool.tile([128, sw + nex, ROW_ELEMS], bf16, tag="G")
            cc = 0
            gseg = []  # per-half compute segment start
            gs = 0
            for hh, Wh, xh in ((0, W0, x0), (1, W1, x1)):
                gseg.append(gs)
                if Wh == 0:
                    continue
                ncols = nblk * Wh + xh
                ni = 128 * ncols
                gcall = nc.gpsimd.dma_gather(
                    G[:, gs:gs + ncols, :],
                    table[base[hh]:base[hh] + 1, :],
                    idx_t[:, cc:cc + 8 * ncols],
                    ni, ni, ROW_ELEMS,
                    single_packet=False,
                    queue_num=call_i % 4,
                )
                add_dep_helper(gcall.ins, anchors[hh].ins, sync=True,
                               reason="gather after table half")
                call_i += 1
                cc += 8 * ncols
                gs += ncols

            # compute per half
            U = [None, None]
            D = [None, None]
            for hh, Wh in ((0, W0), (1, W1)):
                s0 = gseg[hh]
                assert Wh > 0
                U[hh] = epi.tile([128, nblk, HF], f32, tag=f"U{hh}",
                                 name=f"U{hh}")
                D[hh] = epi.tile([128, nblk, H], f32, tag=f"D{hh}",
                                 name=f"D{hh}")
                seg = G[:, s0:s0 + nblk * Wh, :]
                al = seg[:, :, HF:HF + 2 * H].bitcast(f32)  # [128, nW, 8]
                s_t = work.tile([128, nblk * Wh, H], f32, tag="s")
                nc.vector.tensor_tensor(
                    out=s_t[:].rearrange("p (k w) h -> p k w h", k=nblk),
                    in0=al.rearrange("p (k w) h -> p k w h", k=nblk),
                    in1=sk[:, :, HF:HF + H].unsqueeze(2).to_broadcast(
                        [128, nblk, Wh, H]),
                    op=mybir.AluOpType.add)
                e1 = work.tile([128, nblk * Wh, H], f32, tag="e1")
                nc.scalar.activation(out=e1[:], in_=s_t[:],
                                     func=mybir.ActivationFunctionType.Exp,
                                     scale=NEG_SLOPE)
                r_t = work.tile([128, nblk * Wh, H], f32, tag="r")
                nc.scalar.activation(out=r_t[:], in_=s_t[:],
                                     func=mybir.ActivationFunctionType.Relu)
                # e2 -> overwrite s_t (no longer needed)
                nc.scalar.activation(out=s_t[:], in_=r_t[:],
                                     func=mybir.ActivationFunctionType.Exp,
                                     scale=1.0 - NEG_SLOPE)
                Ebf = work.tile([128, nblk * Wh, H], bf16, tag="Eb")
                nc.vector.tensor_tensor(out=Ebf[:], in0=e1[:], in1=s_t[:],
                                        op=mybir.AluOpType.mult)
                M = work.tile([128, nblk * Wh, HF], bf16, tag="M")
                nc.vector.tensor_tensor(
                    out=M[:].rearrange("p w (h f) -> p w h f", h=H),
                    in0=seg[:, :, 0:HF].rearrange("p w (h f) -> p w h f",
                                                  h=H),
                    in1=Ebf[:].unsqueeze(3).to_broadcast(
                        [128, nblk * Wh, H, F]),
                    op=mybir.AluOpType.mult)
                nc.vector.tensor_reduce(
                    out=U[hh][:],
                    in_=M[:].rearrange("p (k w) e -> p k w e",
                                       k=nblk).transpose([0, 1, 3, 2]),
                    axis=mybir.AxisListType.X, op=mybir.AluOpType.add)
                nc.vector.tensor_reduce(
                    out=D[hh][:],
                    in_=Ebf[:].rearrange("p (k w) e -> p k w e",
                                         k=nblk).transpose([0, 1, 3, 2]),
                    axis=mybir.AxisListType.X, op=mybir.AluOpType.add)

            Us = epi.tile([128, nblk, HF], f32, tag="Us")
            nc.vector.tensor_tensor(out=Us[:], in0=U[0][:], in1=U[1][:],
                                    op=mybir.AluOpType.add)
            Ds = epi.tile([128, nblk, H], f32, tag="Ds")
            # Ds = (D0 + EPS) + D1
            nc.vector.scalar_tensor_tensor(
                out=Ds[:], in0=D[0][:], scalar=EPS, in1=D[1][:],
                op0=mybir.AluOpType.add, op1=mybir.AluOpType.add)
            Di = epi.tile([128, nblk, H], f32, tag="Di")
            nc.vector.reciprocal(out=Di[:], in_=Ds[:])
            O = epi.tile([128, nblk, HF], f32, tag="O")
            nc.vector.tensor_tensor(
                out=O[:].rearrange("p k (h f) -> p k h f", h=H),
                in0=Us[:].rearrange("p k (h f) -> p k h f", h=H),
                in1=Di[:].unsqueeze(3).to_broadcast([128, nblk, H, F]),
                op=mybir.AluOpType.mult)
            nc.vector.tensor_tensor(out=O[:], in0=O[:], in1=sk[:, :, 0:HF],
                                    op=mybir.AluOpType.add)
            # ELU(O) = (relu(O) - 1) + exp(-relu(-O))
            R1 = epi.tile([128, nblk, HF], f32, tag="R1")
            nc.scalar.activation(out=R1[:], in_=O[:],
                                 func=mybir.ActivationFunctionType.Relu)
            R2 = epi.tile([128, nblk, HF], f32, tag="R2")
            nc.scalar.activation(out=R2[:], in_=O[:],
                                 func=mybir.ActivationFunctionType.Relu,
                                 scale=-1.0)
            E2 = epi.tile([128, nblk, HF], f32, tag="E2")
            nc.scalar.activation(out=E2[:], in_=R2[:],
                                 func=mybir.ActivationFunctionType.Exp,
                                 scale=-1.0)
            OUT = epi.tile([128, nblk, HF], f32, tag="OUT")
            nc.vector.scalar_tensor_tensor(
                out=OUT[:], in0=R1[:], scalar=-1.0, in1=E2[:],
                op0=mybir.AluOpType.add, op1=mybir.AluOpType.add)
            out_view = out_d[b0 * 128:(b0 + nblk) * 128, :].rearrange(
                "(k p) e -> p k e", k=nblk)
            nc.sync.dma_start(out=out_view, in_=OUT[:])

    nc.compile()
    return nc


def kernel(x, edge_index, W_proj, W_skip, a_src, a_tgt):
    common, per_core = _host_prep(x, edge_index, W_proj, W_skip, a_src, a_tgt)
    key = "prog"
    if key not in _COMPILED:
        _COMPILED[key] = _build_program(common["batches"], common["base"],
                                        common["C_total"], common["n0"])
    nc = _COMPILED[key]

    in_maps = []
    for c in range(N_CORES):
        pc = per_core[c]
        in_maps.append({
            "xTtab": common["xTtab"],
            "xTperm": pc["xTperm"],
            "pack0": common["pack0"],
            "pack2": common["pack2"],
            "idxs": pc["idxs"],
        })
    trace = bool(int(os.environ.get("GAT_TRACE", "0")))
    res = run_bass_kernel_spmd(nc, in_maps, list(range(N_CORES)),
                               trace=trace)
    if trace:
        kernel.last_exec_time_ns = res.exec_time_ns
        kernel.last_mean_exec_time_ns = res.mean_exec_time_ns

    out = np.empty((N_NODES, HF), np.float32)
    for c in range(N_CORES):
        o = res.results[c]["out"]  # [12544, 128] in rank order
        out[per_core[c]["perm"]] = o[:TGT_PER_CORE]
    return out


kernel.last_exec_time_ns = None
kernel.last_mean_exec_time_ns = None


# revision 29
# speedup vs baseline: 1.3166x; 1.3166x over previous
"""GAT layer on 8 Trainium2 NeuronCores (Bass/Tile) — v2.

Strategy (target-per-partition, edge gathers via ANT dma_gather):
  - Nodes split into 2 balanced halves (greedy: each target's edges split
    ~deg/2 per half) laid out as table rows [half0 | sent0 | half1 | sent1].
    Each gather call uses a signed-int16 index window (base row per call,
    idx = row - base in [-32768, 32767]) so one call covers a whole half.
  - Targets sorted globally by max-half count, dealt round-robin to the 8
    cores, so all cores share one tight width schedule W[block, half].
  - Blocks grouped into batches (<=4 blocks, bounded G tile width) with
    batch-uniform widths; one gather call per (batch, half).
  - Table rows (512B): [p bf16(128) | alpha_src f32(8) | pad]. Built by PE
    matmuls (stationary xtt tile f32, moving [W_proj | W_proj@Ablk]).
    Sentinel rows come from a host-solved x column with Wa^T x = -300, so
    exp(s) underflows to 0 for padding slots.
  - Per batch: beta/skip via PE from xTperm; s = alpha + beta; factored
    exp(lrelu) on ACT; E-weighted sum U and denominator D via DVE strided
    reduces; out = U/D + skip, ELU.
"""

import os
import sys

sys.path.insert(0, "/opt/trn_rl_repo")

import numpy as np
from contextlib import ExitStack

import concourse.bass as bass
import concourse.bacc as bacc
import concourse.tile as tile
from concourse import mybir
from concourse._compat import cdiv
from concourse.bass_utils import run_bass_kernel_spmd
from concourse.library_config import mlp

N_NODES = 100000
N_EDGES = 1600000
IN_F = 128
H = 8
F = 16
HF = H * F  # 128
NEG_SLOPE = 0.2
EPS = 1e-16
N_CORES = 8
TGT_PER_CORE = N_NODES // N_CORES  # 12500
N_BLOCKS = cdiv(TGT_PER_CORE, 128)  # 98
TGT_PAD = N_BLOCKS * 128  # 12544
NH = 2
HALF_CAP = 65000
ROW_ELEMS = 256  # bf16 elems per table row (512B)
TABLE_ROWS = N_NODES + 2  # 100002 (two sentinels)
TABLE_ROWS_PAD = cdiv(TABLE_ROWS, 1024) * 1024  # staging writes full groups
ALPHA_SENT = -300.0
GCOL_LIMIT = 64  # max gather-tile columns per batch (SBUF budget)

_COMPILED = {}


def _host_prep(x, edge_index, W_proj, W_skip, a_src, a_tgt):
    x = np.asarray(x, np.float32)
    ei = np.asarray(edge_index)
    src = ei[0].astype(np.int64)
    tgt = ei[1].astype(np.int64)

    Wp = np.asarray(W_proj, np.float32)
    Ws = np.asarray(W_skip, np.float32)
    asr = np.asarray(a_src, np.float32).reshape(H, F)
    atg = np.asarray(a_tgt, np.float32).reshape(H, F)
    # block-diagonal score layouts: Ablk[hf, h] = a_src[h, f'] at hf=h*16+f'
    Ablk = np.zeros((HF, H), np.float32)
    Bblk = np.zeros((HF, H), np.float32)
    for h in range(H):
        Ablk[h * F:(h + 1) * F, h] = asr[h]
        Bblk[h * F:(h + 1) * F, h] = atg[h]
    Wa = Wp @ Ablk  # [128, 8]
    Wb = Wp @ Bblk
    pack0 = np.concatenate([Wp, Wa], axis=1)  # [128, 136]
    pack2 = np.concatenate([Ws, Wb], axis=1)

    # sentinel x column: Wa^T xs = ALPHA_SENT * 1  (min-norm solution)
    xs = np.linalg.lstsq(Wa.T, np.full(H, ALPHA_SENT, np.float64),
                         rcond=None)[0].astype(np.float32)

    # ---- balanced half assignment (per-target even split) ----
    eorder = np.argsort(src, kind="stable")
    t_by_src = tgt[eorder]
    starts = np.searchsorted(src[eorder], np.arange(N_NODES + 1))
    deg_out = np.diff(starts)
    cnt = np.zeros((N_NODES, NH), np.int32)
    hof = np.zeros(N_NODES, np.int8)
    hsize = np.zeros(NH, np.int64)
    for n in np.argsort(-deg_out, kind="stable"):
        ts = t_by_src[starts[n]:starts[n + 1]]
        if len(ts):
            sc = cnt[ts].sum(axis=0).astype(np.float64)
        else:
            sc = hsize.astype(np.float64) * 1e-6
        sc[hsize >= HALF_CAP] = np.inf
        q = int(np.argmin(sc))
        hof[n] = q
        hsize[q] += 1
        if len(ts):
            np.add.at(cnt, (ts, q), 1)
    # refinement sweeps: move nodes between halves when it lowers
    # sum_t max(cnt[t,0], cnt[t,1])
    for _sweep in range(2):
        moved = 0
        for n in np.argsort(-deg_out, kind="stable"):
            ts = t_by_src[starts[n]:starts[n + 1]]
            if not len(ts):
                continue
            q = int(hof[n])
            qn = 1 - q
            if hsize[qn] >= HALF_CAP:
                continue
            c = cnt[ts]  # [k, 2]
            cur = np.maximum(c[:, 0], c[:, 1])
            cq = c[:, q].copy()
            cqn = c[:, qn].copy()
            new = np.maximum(cq - 1, cqn + 1)
            delta = int((new - cur).sum())
            if delta < 0:
                np.add.at(cnt, (ts, q), -1)
                np.add.at(cnt, (ts, qn), 1)
                hof[n] = qn
                hsize[q] -= 1
                hsize[qn] += 1
                moved += 1
        if moved == 0:
            break
    n0 = int(hsize[0])
    n1 = int(hsize[1])
    assert n0 + n1 == N_NODES and max(n0, n1) <= HALF_CAP
    assert n0 + 1 >= (N_NODES + 1 - 32767) - 32768, \
        "half1 window out of int16 range"

    # table rows: [half0 nodes | sent0 | half1 nodes | sent1]
    tabrow = np.empty(N_NODES, np.int64)
    h0_nodes = np.flatnonzero(hof == 0)
    h1_nodes = np.flatnonzero(hof == 1)
    tabrow[h0_nodes] = np.arange(n0)
    tabrow[h1_nodes] = n0 + 1 + np.arange(n1)
    sent_row = [n0, N_NODES + 1]
    base = [max(0, n0 - 32767), N_NODES + 1 - 32767]
    assert -32768 <= 0 - base[0] and sent_row[0] - base[0] <= 32767
    assert -32768 <= (n0 + 1) - base[1] and sent_row[1] - base[1] <= 32767

    # xTtab: x columns in table-row order (bf16), sentinel cols = xs
    import ml_dtypes
    bf = ml_dtypes.bfloat16
    xTtab = np.empty((IN_F, TABLE_ROWS), bf)
    xTtab[:, :n0] = x[h0_nodes].T
    xTtab[:, n0] = xs
    xTtab[:, n0 + 1:n0 + 1 + n1] = x[h1_nodes].T
    xTtab[:, N_NODES + 1] = xs

    # ---- global target ranking & shared width schedule ----
    mx = cnt.max(axis=1)
    rank = np.argsort(-mx, kind="stable")  # rank r -> target node
    rank_of = np.empty(N_NODES, np.int64)
    rank_of[rank] = np.arange(N_NODES)
    cnt_pad = np.vstack([cnt[rank],
                         np.zeros((N_BLOCKS * 1024 - N_NODES, NH), np.int32)])
    Wblk = cnt_pad.reshape(N_BLOCKS, 1024, NH).max(axis=1)  # [98, 2]

    # batches: <=4 blocks, nblk*(W0+W1) <= GCOL_LIMIT
    batches = []  # (b0, nblk, W0, W1, x0, x1)
    b = 0
    while b < N_BLOCKS:
        nblk = 1
        W0, W1 = int(Wblk[b, 0]), int(Wblk[b, 1])
        while (b + nblk < N_BLOCKS and nblk < 4):
            nW0 = max(W0, int(Wblk[b + nblk, 0]))
            nW1 = max(W1, int(Wblk[b + nblk, 1]))
            if (nblk + 1) * (nW0 + nW1) > GCOL_LIMIT:
                break
            W0, W1 = nW0, nW1
            nblk += 1
        # extra all-sentinel column needed only if the stream's last slot
        # (last block, partition 127, last depth) can be a real edge on
        # some core (firmware trims trailing negative idxs)
        j_last = (b + nblk - 1) * 128 + 127
        xs_ = []
        for hh, Wh in ((0, W0), (1, W1)):
            cmax = int(cnt_pad[j_last * 8:j_last * 8 + 8, hh].max())
            xs_.append(1 if (Wh > 0 and cmax >= Wh) else 0)
        batches.append((b, nblk, W0, W1, xs_[0], xs_[1]))
        b += nblk

    # ---- per-core edge slots ----
    core_of = rank_of[tgt] % N_CORES
    j_of = rank_of[tgt] // N_CORES  # 0..12499 within core
    h_of_e = hof[src].astype(np.int64)
    idxval = (tabrow[src] - np.array(base)[h_of_e]).astype(np.int64)

    per_core = []
    for c in range(N_CORES):
        m = core_of == c
        j = j_of[m]
        he = h_of_e[m]
        iv = idxval[m]
        # occurrence index within each (j, h) group
        eo = np.lexsort((np.zeros_like(j), he, j))
        j_s, h_s, iv_s = j[eo], he[eo], iv[eo]
        key = j_s * NH + h_s
        uk, firsts = np.unique(key, return_index=True)
        d_s = np.arange(len(key)) - firsts[np.searchsorted(uk, key)]
        blk_s = j_s // 128
        p_s = j_s % 128

        idx_cols = []
        for bi, (b0, nblk, W0, W1, x0, x1) in enumerate(batches):
            mm_b = (blk_s >= b0) & (blk_s < b0 + nblk)
            for hh, Wh, xh in ((0, W0, x0), (1, W1, x1)):
                if Wh == 0:
                    continue
                arr = np.full((nblk * Wh + xh, 128),
                              sent_row[hh] - base[hh], np.int64)
                mm = mm_b & (h_s == hh)
                if mm.any():
                    col = (blk_s[mm] - b0) * Wh + d_s[mm]
                    arr[col, p_s[mm]] = iv_s[mm]
                flat = arr.reshape(-1)  # j = col*128 + p
                assert flat[-1] >= 0
                wrap = flat.reshape(-1, 16).T  # [16, ni/16]
                idx_cols.append(np.tile(wrap, (8, 1)))
        idxs = np.concatenate(idx_cols, axis=1).astype(np.int16)

        # xTperm: x columns of this core's targets in rank order
        perm = rank[np.arange(TGT_PER_CORE) * N_CORES + c]  # j -> node
        xTp = np.zeros((IN_F, TGT_PAD), bf)
        xTp[:, :TGT_PER_CORE] = x[perm].T
        per_core.append(dict(idxs=np.ascontiguousarray(idxs),
                             xTperm=np.ascontiguousarray(xTp), perm=perm))

    pack0 = pack0.astype(bf)
    pack2 = pack2.astype(bf)
    common = dict(xTtab=xTtab, pack0=pack0, pack2=pack2,
                  batches=batches, base=base, n0=n0,
                  C_total=per_core[0]["idxs"].shape[1])
    for pc in per_core:
        assert pc["idxs"].shape[1] == common["C_total"]
    return common, per_core


def _build_program(batches, base, C_total, n0):
    nc = bacc.Bacc("TRN2", debug=False, num_devices=N_CORES,
                   num_swdge_queues=4)
    f32 = mybir.dt.float32
    bf16 = mybir.dt.bfloat16
    i16 = mybir.dt.int16

    xTtab_d = nc.dram_tensor("xTtab", [IN_F, TABLE_ROWS], bf16,
                             kind="ExternalInput").ap()
    xTperm_d = nc.dram_tensor("xTperm", [IN_F, TGT_PAD], bf16,
                              kind="ExternalInput").ap()
    pack0_d = nc.dram_tensor("pack0", [IN_F, HF + H], bf16,
                             kind="ExternalInput").ap()
    pack2_d = nc.dram_tensor("pack2", [IN_F, HF + H], bf16,
                             kind="ExternalInput").ap()
    idxs_d = nc.dram_tensor("idxs", [128, C_total], i16,
                            kind="ExternalInput").ap()
    out_d = nc.dram_tensor("out", [TGT_PAD, HF], f32,
                           kind="ExternalOutput").ap()
    table = nc.dram_tensor("table", [TABLE_ROWS_PAD, ROW_ELEMS], bf16).ap()

    with tile.TileContext(nc) as tc, ExitStack() as ctx:
        consts = ctx.enter_context(tc.tile_pool(name="consts", bufs=1))
        stg = ctx.enter_context(tc.tile_pool(name="stg", bufs=3))
        gpool = ctx.enter_context(tc.tile_pool(name="gpool", bufs=3))
        work = ctx.enter_context(tc.tile_pool(name="work", bufs=2))
        epi = ctx.enter_context(tc.tile_pool(name="epi", bufs=2))
        psum = ctx.enter_context(tc.tile_pool(name="psum", bufs=3,
                                              space="PSUM"))
        psum2 = ctx.enter_context(tc.tile_pool(name="psum2", bufs=2,
                                               space="PSUM"))
        idxp = ctx.enter_context(tc.tile_pool(name="idxp", bufs=2))

        nc.gpsimd.load_library(mlp)

        pack0 = consts.tile([IN_F, HF + H], bf16)
        nc.sync.dma_start(out=pack0[:], in_=pack0_d[:])
        pack2 = consts.tile([IN_F, HF + H], bf16)
        nc.sync.dma_start(out=pack2[:], in_=pack2_d[:])

        # --- Phase B: build table (groups of 16 row-tiles = 2048 rows) ---
        from concourse.tile_rust import add_dep_helper
        tab_stores = [[], []]  # store insts overlapping each half's rows
        GR = 1024
        NGRP = TABLE_ROWS_PAD // GR
        for g in range(NGRP):
            r0 = g * GR
            ncols = min(GR, TABLE_ROWS - r0)  # real columns to load
            nt = cdiv(ncols, 128)
            xtt = stg.tile([IN_F, GR], bf16, tag="xtt")
            nc.sync.dma_start(out=xtt[:, :ncols],
                              in_=xTtab_d[:, r0:r0 + ncols])
            rows_s = stg.tile([128, GR // 128, ROW_ELEMS], bf16, tag="rows")
            for t in range(nt):
                nr = min(128, ncols - t * 128)
                pa = psum.tile([128, HF + H], f32, space="PSUM", tag="pa")
                nc.tensor.matmul(out=pa[:nr],
                                 lhsT=xtt[:, t * 128:t * 128 + nr],
                                 rhs=pack0[:], start=True, stop=True)
                if t % 2 == 0:
                    nc.scalar.activation(
                        out=rows_s[:nr, t, 0:HF], in_=pa[:nr, 0:HF],
                        func=mybir.ActivationFunctionType.Copy)
                else:
                    nc.vector.tensor_copy(out=rows_s[:nr, t, 0:HF],
                                          in_=pa[:nr, 0:HF])
                nc.vector.tensor_copy(
                    out=rows_s[:nr, t, HF:HF + 2 * H].bitcast(f32),
                    in_=pa[:nr, HF:HF + H])
            # store: table row r0 + t*128 + p <- rows_s[p, t, :]
            tab_view = table[r0:r0 + GR, :].rearrange(
                "(t p) e -> p t e", t=GR // 128)
            st = nc.sync.dma_start(out=tab_view, in_=rows_s[:])
            if r0 <= n0:  # overlaps window A rows [0, n0]
                tab_stores[0].append(st)
            if r0 + GR > n0 + 1:  # overlaps window B rows [n0+1, end]
                tab_stores[1].append(st)

        # table-half-complete anchors: the gather in_ap only declares one
        # row, so the dep tracker can't see the real read range — add
        # explicit edges table-stores -> anchor -> gather.
        anchors = []
        for hh in range(NH):
            a = nc.sync.nop()
            for st in tab_stores[hh]:
                add_dep_helper(a.ins, st.ins, sync=True,
                               reason="table half complete")
            anchors.append(a)

        # --- Phase C: batched gathers + compute ---
        col_off = 0
        call_i = 0
        for bi, (b0, nblk, W0, W1, x0, x1) in enumerate(batches):
            sw = nblk * (W0 + W1)
            # beta / skip
            xp = stg.tile([IN_F, nblk * 128], bf16, tag="xp")
            nc.sync.dma_start(
                out=xp[:], in_=xTperm_d[:, b0 * 128:(b0 + nblk) * 128])
            sk = epi.tile([128, nblk, HF + H], f32, tag="sk")
            for k in range(nblk):
                sk_ps = psum2.tile([128, HF + H], f32, space="PSUM",
                                   tag="skps")
                nc.tensor.matmul(out=sk_ps[:],
                                 lhsT=xp[:, k * 128:(k + 1) * 128],
                                 rhs=pack2[:], start=True, stop=True)
                nc.scalar.activation(out=sk[:, k, :], in_=sk_ps[:],
                                     func=mybir.ActivationFunctionType.Copy)

            # idx + gathers (optional extra all-sentinel column per call)
            nex = x0 + x1
            Cb = (nblk * (W0 + W1) + nex) * 8
            idx_t = idxp.tile([128, Cb], i16, tag="idxg")
            nc.sync.dma_start(out=idx_t[:],
                              in_=idxs_d[:, col_off:col_off + Cb])
            col_off += Cb

            G = gpool.tile([128, sw + nex, ROW_ELEMS], bf16, tag="G")
            cc = 0
            gseg = []  # per-half compute segment start
            gs = 0
            for hh, Wh, xh in ((0, W0, x0), (1, W1, x1)):
                gseg.append(gs)
                if Wh == 0:
                    continue
                ncols = nblk * Wh + xh
                ni = 128 * ncols
                gcall = nc.gpsimd.dma_gather(
                    G[:, gs:gs + ncols, :],
                    table[base[hh]:base[hh] + 1, :],
                    idx_t[:, cc:cc + 8 * ncols],
                    ni, ni, ROW_ELEMS,
                    single_packet=False,
                    queue_num=call_i % 4,
                )
                add_dep_helper(gcall.ins, anchors[hh].ins, sync=True,
                               reason="gather after table half")
                call_i += 1
                cc += 8 * ncols
                gs += ncols

            # compute per half
            U = [None, None]
            D = [None, None]
            for hh, Wh in ((0, W0), (1, W1)):
                s0 = gseg[hh]
                assert Wh > 0
                U[hh] = epi.tile([128, nblk, HF], f32, tag=f"U{hh}",
                                 name=f"U{hh}")
                D[hh] = epi.tile([128, nblk, H], f32, tag=f"D{hh}",
                                 name=f"D{hh}")
                seg = G[:, s0:s0 + nblk * Wh, :]
                al = seg[:, :, HF:HF + 2 * H].bitcast(f32)  # [128, nW, 8]
                s_t = work.tile([128, nblk * Wh, H], f32, tag="s")
                nc.vector.tensor_tensor(
                    out=s_t[:].rearrange("p (k w) h -> p k w h", k=nblk),
                    in0=al.rearrange("p (k w) h -> p k w h", k=nblk),
                    in1=sk[:, :, HF:HF + H].unsqueeze(2).to_broadcast(
                        [128, nblk, Wh, H]),
                    op=mybir.AluOpType.add)
                e1 = work.tile([128, nblk * Wh, H], f32, tag="e1")
                nc.scalar.activation(out=e1[:], in_=s_t[:],
                                     func=mybir.ActivationFunctionType.Exp,
                                     scale=NEG_SLOPE)
                r_t = work.tile([128, nblk * Wh, H], f32, tag="r")
                nc.scalar.activation(out=r_t[:], in_=s_t[:],
                                     func=mybir.ActivationFunctionType.Relu)
                # e2 -> overwrite s_t (no longer needed)
                nc.scalar.activation(out=s_t[:], in_=r_t[:],
                                     func=mybir.ActivationFunctionType.Exp,
                                     scale=1.0 - NEG_SLOPE)
                Ebf = work.tile([128, nblk * Wh, H], bf16, tag="Eb")
                nc.vector.tensor_tensor(out=Ebf[:], in0=e1[:], in1=s_t[:],
                                        op=mybir.AluOpType.mult)
                M = work.tile([128, nblk * Wh, HF], bf16, tag="M")
                nc.vector.tensor_tensor(
                    out=M[:].rearrange("p w (h f) -> p w h f", h=H),
                    in0=seg[:, :, 0:HF].rearrange("p w (h f) -> p w h f",
                                                  h=H),
                    in1=Ebf[:].unsqueeze(3).to_broadcast(
                        [128, nblk * Wh, H, F]),
                    op=mybir.AluOpType.mult)
                nc.vector.tensor_reduce(
                    out=U[hh][:],
                    in_=M[:].rearrange("p (k w) e -> p k w e",
                                       k=nblk).transpose([0, 1, 3, 2]),
                    axis=mybir.AxisListType.X, op=mybir.AluOpType.add)
                nc.vector.tensor_reduce(
                    out=D[hh][:],
                    in_=Ebf[:].rearrange("p (k w) e -> p k w e",
                                         k=nblk).transpose([0, 1, 3, 2]),
                    axis=mybir.AxisListType.X, op=mybir.AluOpType.add)

            Us = epi.tile([128, nblk, HF], f32, tag="Us")
            nc.vector.tensor_tensor(out=Us[:], in0=U[0][:], in1=U[1][:],
                                    op=mybir.AluOpType.add)
            Ds = epi.tile([128, nblk, H], f32, tag="Ds")
            # Ds = (D0 + EPS) + D1
            nc.vector.scalar_tensor_tensor(
                out=Ds[:], in0=D[0][:], scalar=EPS, in1=D[1][:],
                op0=mybir.AluOpType.add, op1=mybir.AluOpType.add)
            Di = epi.tile([128, nblk, H], f32, tag="Di")
            nc.vector.reciprocal(out=Di[:], in_=Ds[:])
            O = epi.tile([128, nblk, HF], f32, tag="O")
            nc.vector.tensor_tensor(
                out=O[:].rearrange("p k (h f) -> p k h f", h=H),
                in0=Us[:].rearrange("p k (h f) -> p k h f", h=H),
                in1=Di[:].unsqueeze(3).to_broadcast([128, nblk, H, F]),
                op=mybir.AluOpType.mult)
            nc.vector.tensor_tensor(out=O[:], in0=O[:], in1=sk[:, :, 0:HF],
                                    op=mybir.AluOpType.add)
            # ELU(O) = (relu(O) - 1) + exp(-relu(-O))
            R1 = epi.tile([128, nblk, HF], f32, tag="R1")
            nc.scalar.activation(out=R1[:], in_=O[:],
                                 func=mybir.ActivationFunctionType.Relu)
            R2 = epi.tile([128, nblk, HF], f32, tag="R2")
            nc.scalar.activation(out=R2[:], in_=O[:],
                                 func=mybir.ActivationFunctionType.Relu,
                                 scale=-1.0)
            E2 = epi.tile([128, nblk, HF], f32, tag="E2")
            nc.scalar.activation(out=E2[:], in_=R2[:],
                                 func=mybir.ActivationFunctionType.Exp,
                                 scale=-1.0)
            OUT = epi.tile([128, nblk, HF], f32, tag="OUT")
            nc.vector.scalar_tensor_tensor(
                out=OUT[:], in0=R1[:], scalar=-1.0, in1=E2[:],
                op0=mybir.AluOpType.add, op1=mybir.AluOpType.add)
            out_view = out_d[b0 * 128:(b0 + nblk) * 128, :].rearrange(
                "(k p) e -> p k e", k=nblk)
            nc.sync.dma_start(out=out_view, in_=OUT[:])

    nc.compile()
    return nc


def kernel(x, edge_index, W_proj, W_skip, a_src, a_tgt):
    common, per_core = _host_prep(x, edge_index, W_proj, W_skip, a_src, a_tgt)
    key = "prog"
    if key not in _COMPILED:
        _COMPILED[key] = _build_program(common["batches"], common["base"],
                                        common["C_total"], common["n0"])
    nc = _COMPILED[key]

    in_maps = []
    for c in range(N_CORES):
        pc = per_core[c]
        in_maps.append({
            "xTtab": common["xTtab"],
            "xTperm": pc["xTperm"],
            "pack0": common["pack0"],
            "pack2": common["pack2"],
            "idxs": pc["idxs"],
        })
    trace = bool(int(os.environ.get("GAT_TRACE", "0")))
    res = run_bass_kernel_spmd(nc, in_maps, list(range(N_CORES)),
                               trace=trace)
    if trace:
        kernel.last_exec_time_ns = res.exec_time_ns
        kernel.last_mean_exec_time_ns = res.mean_exec_time_ns

    out = np.empty((N_NODES, HF), np.float32)
    for c in range(N_CORES):
        o = res.results[c]["out"]  # [12544, 128] in rank order
        out[per_core[c]["perm"]] = o[:TGT_PER_CORE]
    return out


kernel.last_exec_time_ns = None
kernel.last_mean_exec_time_ns = None
